# revision 30
# baseline (speedup 1.0000x reference)
"""GCN (single GCNConv + Cox head) Trainium2 Bass kernel, 8-core SPMD.

Math (per reference):
    src,dst += self loops;  deg = indegree(dst);  dinv = deg^-1/2
    agg[d]  = sum_e 1[dst_e = d] * (dinv[src_e] * dinv[dst_e] * x[src_e])
    out     = relu(agg @ W.T + b) @ w_reg.T + b_reg

Distribution: destination-sharded over 8 cores (12500 dst nodes each), no
collectives — each core gets host-staged per-edge row tables and writes its
output shard; the host concatenates shards.

Design: every edge slot is streamed by HWDGE on the SP queue only (no SWDGE
gather — SWDGE descriptor processing stalls the shared DMA engines at
~2 GB/s; and no DMA issues on the ACT queue — a blocked ACTIVATE would
delay stream issues queued behind it). The full per-edge norm
dinv[src]*dinv[dst] is folded into the streamed fp16 row on the host, so
the scatter one-hot is exactly 0/1 and psum comes out pre-scaled. Matmul
orientation puts features on psum partitions:
    psum[F, dstW] += msg[slot, F].T @ onehot[slot, dstW]
so accT is feat-major with a plain chunked copy (no PE transpose, no ACT
scale). dst windows are W=64 wide (env GCN_W): halves one-hot size and
matmul moving time vs 128. Eight W-windows accumulate into one psum bank
[128, 512]; a single DVE cast per bank lands them in accT. Phase 2
(W^T @ accT chunk, relu+bias, cox row) runs as a two-stage pipeline one
bank behind the scatter stream so no PE-queue entry waits on DVE/ACT work.

The one-hot is generated ON-CHIP by gpsimd local_scatter (dst[:]=0;
dst[:,idx]=1 per partition, 16 batches per call) from a [128, B] int16
index table — ~55us on the otherwise-idle Pool engine instead of 6.4 MB
of fp8 DMA. (tensor_scalar is_equal was tried first: ~2.7us/batch custom
DVE ucode — dead end. GCN_MIX=0,1 falls back to DMA'd fp8 one-hots.)
Remaining HBM traffic is ~25.9 MB/core, sustained at ~425 GB/s.
"""

import os
import time
import numpy as np

N_CORES = 8
CNB_CAP = int(os.environ.get("GCN_CNB", "48"))  # batches per stream-DMA group
CH = 512          # phase-2 column chunk == psum bank width


class Plan:
    def __init__(self, n_feat, w, nblk, nbk, batch_base, groups, npad):
        self.F = n_feat
        self.W = w
        self.NBLK = nblk
        self.NBK = nbk                  # [nblk] batches per block
        self.BB = batch_base            # [nblk+1] cumsum of NBK
        self.GROUPS = groups            # (k0, glen, b0, cnb)
        self.B = int(batch_base[-1])    # total batches per core
        self.NPAD = npad
        self.in_maps = []
        self.mix = (0, 1)
        self.any_gen = False
        self.any_dma = True


def make_plan(x, edge_index, W, b, w_reg, b_reg, n_cores=N_CORES):
    import concourse.mybir as _mybir
    ohnp = _mybir.dt.np(_mybir.dt.float8e4)

    WN = int(os.environ.get("GCN_W", "64"))   # dst window width
    wsh = WN.bit_length() - 1
    assert (1 << wsh) == WN

    x = np.asarray(x, dtype=np.float32)
    N, F = x.shape
    ns = N // n_cores
    assert ns * n_cores == N
    nblk = (ns + WN - 1) // WN
    npad = nblk * WN

    src = np.asarray(edge_index[0], dtype=np.int64)
    dst = np.asarray(edge_index[1], dtype=np.int64)
    deg = (np.bincount(dst, minlength=N) + 1).astype(np.float64)
    dinv = (1.0 / np.sqrt(deg)).astype(np.float32)
    xs = x * dinv[:, None]  # fold dinv[src] into all rows once

    # per-core edge lists (with self loops), sorted by dst block
    cores = []
    cnts = np.zeros((n_cores, nblk), dtype=np.int64)
    for c in range(n_cores):
        lo, hi = c * ns, (c + 1) * ns
        m = (dst >= lo) & (dst < hi)
        s_c = np.concatenate([src[m], np.arange(lo, hi)])
        d_c = np.concatenate([dst[m] - lo, np.arange(ns)])
        nd = dinv[np.concatenate([dst[m], np.arange(lo, hi)])]  # dinv[dst_e]
        blk = d_c >> wsh
        order = np.argsort(blk, kind="stable")
        s_c, d_c, nd, blk = s_c[order], d_c[order], nd[order], blk[order]
        cnts[c] = np.bincount(blk, minlength=nblk)
        cores.append((s_c, (d_c & (WN - 1)).astype(np.int64), nd, blk))

    # shared batch structure: per-block batch count = max over cores,
    # rounded up to even so every group's batch count is even (local_scatter
    # sub-calls must have an even index count)
    nbk = np.maximum(1, -(-cnts // 128)).max(axis=0)
    nbk = nbk + (nbk % 2)
    batch_base = np.concatenate([[0], np.cumsum(nbk)])

    # greedy group packing (shared across cores): small first group so the
    # PE starts early, tapered last groups so the PE backlog at DMA-end is
    # short (PE and DMA rates are nearly equal in steady state).
    Ball = int(batch_base[-1])
    groups = []
    k0 = 0
    while k0 < nblk:
        cnb = 0
        k1 = k0
        rem = Ball - int(batch_base[k0])
        if not groups:
            cap = max(int(nbk[0]), CNB_CAP // 4)
        elif rem > 2 * CNB_CAP:
            cap = CNB_CAP
        elif rem > 48:
            cap = max(int(nbk[k1]), 32)
        elif rem > 24:
            cap = max(int(nbk[k1]), 16)
        else:
            cap = max(int(nbk[k1]), 8)
        while k1 < nblk and cnb + nbk[k1] <= cap:
            cnb += int(nbk[k1])
            k1 += 1
        groups.append((k0, k1 - k0, int(batch_base[k0]), cnb))
        k0 = k1

    plan = Plan(F, WN, nblk, nbk, batch_base, groups, npad)

    mix = os.environ.get("GCN_MIX", "1,1")
    num, den = (int(v) for v in mix.split(","))
    plan.mix = (num, den)
    gen_flags = [(g % den) < num for g in range(len(groups))]
    plan.gen_flags = gen_flags
    plan.any_gen = any(gen_flags)
    plan.any_dma = not all(gen_flags)

    # per-block group-derived arrays for edge row addressing
    b0_of_blk = np.zeros(nblk, dtype=np.int64)
    cnb_of_blk = np.zeros(nblk, dtype=np.int64)
    for (k0, glen, b0, cnb) in groups:
        b0_of_blk[k0:k0 + glen] = b0
        cnb_of_blk[k0:k0 + glen] = cnb

    consts = {
        "wt": np.ascontiguousarray(np.asarray(W, np.float32).T).astype(
            np.float16),
        "bvec": np.asarray(b, np.float32).reshape(F, 1),
        "wreg": np.ascontiguousarray(
            np.asarray(w_reg, np.float32).T).astype(np.float16),
        "breg": np.asarray(b_reg, np.float32).reshape(1, 1),
    }

    Btot = plan.B
    for c in range(n_cores):
        s_c, rel_c, nd_c, blk_c = cores[c]
        bstart = np.searchsorted(blk_c, np.arange(nblk))
        q = np.arange(len(s_c)) - bstart[blk_c]       # pos within block
        ce = q >> 7                                    # batch within block
        pe = q & 127                                   # partition (slot)
        assert int((ce - nbk[blk_c] + 1).max()) <= 0, "block overflow"
        Be = batch_base[blk_c] + ce                    # global batch idx
        Re = (128 * b0_of_blk[blk_c] + pe * cnb_of_blk[blk_c]
              + (batch_base[blk_c] - b0_of_blk[blk_c]) + ce)

        xg = np.zeros((Btot * 128, F), dtype=np.float16)
        xg[Re] = (xs[s_c] * nd_c[:, None]).astype(np.float16)

        im = {"xg": xg, **consts}
        if plan.any_gen:
            # local_scatter indices: sub-calls cover 16 batches starting at
            # each group's first batch; the target column of slot p is
            # (offset-within-sub-call)*WN + rel. -1 (pad) is ignored by the
            # instruction.
            drl = np.full((128, Btot), -1, dtype=np.int16)
            drl[pe, Be] = ((Be - b0_of_blk[blk_c]) % 16) * WN + rel_c
            im["drl"] = drl
        if plan.any_dma:
            oh = np.zeros((128, Btot * WN), dtype=ohnp)
            oh[pe, Be * WN + rel_c] = 1.0
            im["oh"] = oh
        plan.in_maps.append(im)
    return plan


# ---------------------------------------------------------------------------
def build_nc(plan):
    import concourse.bacc as bacc
    import concourse.mybir as mybir
    import concourse.tile as tile

    f32 = mybir.dt.float32
    f16 = mybir.dt.float16
    oh8 = mybir.dt.float8e4
    F, WN, NBLK, NPAD, Btot = plan.F, plan.W, plan.NBLK, plan.NPAD, plan.B
    NBK, BB = plan.NBK, plan.BB
    BPB = CH // WN                      # W-blocks per psum bank

    dve_n = int(os.environ.get("GCN_DVE", "2"))
    pool_n = int(os.environ.get("GCN_POOL", "1"))

    nc = bacc.Bacc("TRN2", target_bir_lowering=False, debug=False)

    xg = nc.dram_tensor("xg", [Btot * 128, F], f16, kind="ExternalInput").ap()
    drl = (nc.dram_tensor("drl", [128, Btot], mybir.dt.int16,
                          kind="ExternalInput").ap()
           if plan.any_gen else None)
    oh = (nc.dram_tensor("oh", [128, Btot * WN], oh8,
                         kind="ExternalInput").ap()
          if plan.any_dma else None)
    wt = nc.dram_tensor("wt", [F, F], f16, kind="ExternalInput").ap()
    bvec = nc.dram_tensor("bvec", [F, 1], f32, kind="ExternalInput").ap()
    wreg = nc.dram_tensor("wreg", [F, 1], f16, kind="ExternalInput").ap()
    breg = nc.dram_tensor("breg", [1, 1], f32, kind="ExternalInput").ap()
    out = nc.dram_tensor("out", [1, NPAD], f32, kind="ExternalOutput").ap()

    with tile.TileContext(nc) as tc:
        with (
            tc.tile_pool(name="const", bufs=1) as cpool,
            tc.tile_pool(name="stream", bufs=5) as spool,
            tc.tile_pool(name="ohp", bufs=6) as ogpool,
            tc.tile_pool(name="ps", bufs=5, space="PSUM") as pspool,
            tc.tile_pool(name="ph2", bufs=2, space="PSUM") as ph2pool,
            tc.tile_pool(name="po", bufs=1, space="PSUM") as popool,
            tc.tile_pool(name="hrelu", bufs=2) as hpool,
        ):
            wt_sb = cpool.tile([F, F], f16)
            b_sb = cpool.tile([F, 1], f32)
            wreg_sb = cpool.tile([F, 1], f16)
            breg_sb = cpool.tile([1, 1], f32)
            accT = cpool.tile([128, NPAD], f16)
            out_sb = cpool.tile([1, NPAD], f32)

            for sb, dr in ((wt_sb, wt), (b_sb, bvec), (wreg_sb, wreg),
                           (breg_sb, breg)):
                nc.scalar.dma_start(out=sb[:], in_=dr[:])

            if plan.any_gen:
                drl_sb = cpool.tile([128, Btot], mybir.dt.int16)
                nc.sync.dma_start(out=drl_sb[:], in_=drl[:])
                ones_sb = cpool.tile([128, 16], f16)
                nc.vector.memset(ones_sb[:], 1.0)

            # phase-2 two-stage pipeline, decoupled from the scatter stream:
            # stage A (W^T @ accT chunk) fires one bank after the chunk's
            # CAST; stage B (relu result -> cox row) one bank after stage A —
            # so each PE-queue entry's dependency is already satisfied when
            # it reaches the head (the in-order PE queue never stalls on
            # DVE/ACT work).
            pend_a = []  # chunks awaiting stage A
            pend_b = []  # (c0, c1, hr) awaiting stage B

            def phase2_a(c0, c1):
                cw = c1 - c0
                ph = ph2pool.tile([128, CH], f32)
                hr = hpool.tile([128, CH], f16)
                nc.tensor.matmul(ph[:, :cw], lhsT=wt_sb[:],
                                 rhs=accT[:, c0:c1], start=True, stop=True)
                nc.scalar.activation(hr[:, :cw], ph[:, :cw],
                                     mybir.ActivationFunctionType.Relu,
                                     bias=b_sb[:, :1])
                return hr

            out_done = [0]

            def phase2_b(c0, c1, hr):
                cw = c1 - c0
                po = popool.tile([1, CH], f32)
                nc.tensor.matmul(po[:, :cw], lhsT=wreg_sb[:], rhs=hr[:, :cw],
                                 start=True, stop=True)
                nc.scalar.activation(out_sb[:, c0:c1], po[:, :cw],
                                     mybir.ActivationFunctionType.Identity,
                                     bias=breg_sb[:, :1])
                # stream the finished half of the output row early so the
                # final out-DMA is small
                if out_done[0] == 0 and c1 >= NPAD // 2:
                    nc.scalar.dma_start(out=out[:, :c1], in_=out_sb[:, :c1])
                    out_done[0] = c1

            def pump_phase2(final=False):
                while pend_b and (final or len(pend_b) > 1):
                    phase2_b(*pend_b.pop(0))
                    if final:  # drain: issue remaining stage As first so
                        break  # they pipeline ahead of the B chain
                while pend_a and (final or len(pend_a) > 1):
                    c0, c1 = pend_a.pop(0)
                    pend_b.append((c0, c1, phase2_a(c0, c1)))
                while pend_b and final:
                    phase2_b(*pend_b.pop(0))

            done_cols = 0
            ps = None
            for gi, (k0, glen, b0, cnb) in enumerate(plan.GROUPS):
                gen = plan.gen_flags[gi]
                st = spool.tile([128, CNB_CAP * F], f16, tag="st")
                q = nc.sync
                q.dma_start(
                    out=st[:, :cnb * F].rearrange("p (c f) -> p c f", f=F),
                    in_=xg[128 * b0:128 * (b0 + cnb), :].rearrange(
                        "(p c) f -> p c f", p=128),
                )
                if gen:
                    ot = ogpool.tile([128, CNB_CAP * WN], f16, tag="ot")
                    j0 = 0
                    while j0 < cnb:
                        sub = min(16, cnb - j0)
                        nc.gpsimd.local_scatter(
                            out_ap=ot[:, j0 * WN:(j0 + sub) * WN],
                            data_ap=ones_sb[:, :sub],
                            idxs_ap=drl_sb[:, b0 + j0:b0 + j0 + sub],
                            channels=128,
                            num_elems=sub * WN,
                            num_idxs=sub,
                        )
                        j0 += sub
                else:
                    ot = ogpool.tile([128, CNB_CAP * WN], oh8, tag="ot")
                    q2 = nc.sync
                    q2.dma_start(out=ot[:, :cnb * WN],
                                 in_=oh[:, b0 * WN:(b0 + cnb) * WN])

                for i in range(glen):
                    k = k0 + i
                    nb = int(NBK[k])
                    loff = int(BB[k]) - b0
                    kb = k % BPB        # window within psum bank
                    if kb == 0:
                        ps = pspool.tile([128, CH], f32)
                    for cc in range(nb):
                        j = loff + cc
                        nc.tensor.matmul(ps[:, kb * WN:(kb + 1) * WN],
                                         lhsT=st[:, j * F:(j + 1) * F],
                                         rhs=ot[:, j * WN:(j + 1) * WN],
                                         start=(cc == 0), stop=(cc == nb - 1))
                    if kb == BPB - 1 or k == NBLK - 1:
                        cb0 = (k - kb) * WN
                        cw = (kb + 1) * WN
                        nc.vector.tensor_copy(accT[:, cb0:cb0 + cw],
                                              ps[:, :cw])
                        avail = (k + 1) * WN
                        while done_cols + CH <= avail or (k == NBLK - 1
                                                         and done_cols < NPAD):
                            c1 = min(done_cols + CH, NPAD)
                            if c1 == NPAD and c1 - done_cols > CH // 2:
                                # split the last chunk: halves the length of
                                # the final serial phase-2 dependency chain
                                mid = done_cols + (c1 - done_cols) // 2
                                pend_a.append((done_cols, mid))
                                pend_a.append((mid, c1))
                            else:
                                pend_a.append((done_cols, c1))
                            done_cols = c1
                        pump_phase2(final=(k == NBLK - 1))

            nc.sync.dma_start(out=out[:, out_done[0]:],
                              in_=out_sb[:, out_done[0]:])

    nc.compile()
    return nc


# ---------------------------------------------------------------------------
_CACHE = {}


def _ensure_ntff_hook():
    try:
        from antenv.axon_hooks import get_axon_ntff_profile_hook  # noqa: F401
        return
    except ImportError:
        pass
    import sys
    import types
    import antenv
    mod = types.ModuleType("antenv.axon_hooks")
    mod._hook = None
    mod.set_axon_ntff_profile_hook = lambda h: setattr(mod, "_hook", h)
    mod.get_axon_ntff_profile_hook = lambda: mod._hook
    sys.modules["antenv.axon_hooks"] = mod
    antenv.axon_hooks = mod
    try:
        from trn_agent_boot.trn_boot import _ntff_profile_via_ctypes
        mod._hook = _ntff_profile_via_ctypes("/opt/axon/libaxon_pjrt.so")
    except Exception:
        pass


def _run(plan, nc, trace=False):
    import concourse.bass_utils as bu
    if trace:
        _ensure_ntff_hook()
        bu.upload_artifacts = lambda tmpdir: tmpdir  # no egress here
    core_ids = list(range(len(plan.in_maps)))
    res = bu.run_bass_kernel_spmd(nc, plan.in_maps, core_ids, trace=trace)
    return res


def kernel(x, edge_index, W, b, w_reg, b_reg):
    trace = bool(os.environ.get("GCN_TRACE"))

    plan = make_plan(x, edge_index, W, b, w_reg, b_reg)
    key = (plan.B, plan.W, CNB_CAP, tuple(plan.NBK.tolist()), plan.mix,
           os.environ.get("GCN_DVE"), os.environ.get("GCN_POOL"))
    if key not in _CACHE:
        _CACHE[key] = build_nc(plan)
    nc = _CACHE[key]

    res = None
    for attempt in range(3):
        try:
            res = _run(plan, nc, trace=trace)
            break
        except Exception:
            # transient device errors (e.g. NRT exec-unit resets) recover on
            # a fresh attempt; re-raise only if persistent
            if attempt == 2:
                raise
            time.sleep(5.0)
    kernel.last_exec_ns = res.exec_time_ns
    kernel.last_profile = res.profile_json

    N = np.asarray(x).shape[0]
    ns = N // len(plan.in_maps)
    shards = [res.results[c]["out"][0, :ns] for c in range(len(plan.in_maps))]
    return np.concatenate(shards).reshape(N, 1).astype(np.float32)


kernel.last_exec_ns = None
kernel.last_profile = None


# revision 31
# speedup vs baseline: 1.0628x; 1.0628x over previous
"""GCN (single GCNConv + Cox head) Trainium2 Bass kernel, 8-core SPMD.

Math (per reference):
    src,dst += self loops;  deg = indegree(dst);  dinv = deg^-1/2
    agg[d]  = sum_e 1[dst_e = d] * (dinv[src_e] * dinv[dst_e] * x[src_e])
    out     = relu(agg @ W.T + b) @ w_reg.T + b_reg

Distribution: destination-sharded over 8 cores (12500 dst nodes each), no
collectives — each core gets host-staged per-edge row tables and writes its
output shard; the host concatenates shards.

Design: every edge slot is streamed by HWDGE on the SP queue only (no SWDGE
gather — SWDGE descriptor processing stalls the shared DMA engines at
~2 GB/s; and no DMA issues on the ACT queue — a blocked ACTIVATE would
delay stream issues queued behind it). The full per-edge norm
dinv[src]*dinv[dst] is folded into the streamed fp16 row on the host, so
the scatter one-hot is exactly 0/1 and psum comes out pre-scaled. Matmul
orientation puts features on psum partitions:
    psum[F, dstW] += msg[slot, F].T @ onehot[slot, dstW]
so accT is feat-major with a plain chunked copy (no PE transpose, no ACT
scale). dst windows are W=64 wide (env GCN_W): halves one-hot size and
matmul moving time vs 128. Eight W-windows accumulate into one psum bank
[128, 512]; a single DVE cast per bank lands them in accT. Phase 2
(W^T @ accT chunk, relu+bias, cox row) runs as a two-stage pipeline one
bank behind the scatter stream so no PE-queue entry waits on DVE/ACT work.

The one-hot is generated ON-CHIP by gpsimd local_scatter (dst[:]=0;
dst[:,idx]=1 per partition, 16 batches per call) from a [128, B] int16
index table — ~55us on the otherwise-idle Pool engine instead of 6.4 MB
of fp8 DMA. (tensor_scalar is_equal was tried first: ~2.7us/batch custom
DVE ucode — dead end. GCN_MIX=0,1 falls back to DMA'd fp8 one-hots.)
Remaining HBM traffic is ~25.9 MB/core, sustained at ~425 GB/s.
"""

import os
import time
import numpy as np

N_CORES = 8
CNB_CAP = int(os.environ.get("GCN_CNB", "48"))  # batches per stream-DMA group
CH = 512          # phase-2 column chunk == psum bank width


class Plan:
    def __init__(self, n_feat, w, nblk, nbk, batch_base, groups, npad):
        self.F = n_feat
        self.W = w
        self.NBLK = nblk
        self.NBK = nbk                  # [nblk] batches per block
        self.BB = batch_base            # [nblk+1] cumsum of NBK
        self.GROUPS = groups            # (k0, glen, b0, cnb)
        self.B = int(batch_base[-1])    # total batches per core
        self.NPAD = npad
        self.in_maps = []
        self.mix = (0, 1)
        self.any_gen = False
        self.any_dma = True


def make_plan(x, edge_index, W, b, w_reg, b_reg, n_cores=N_CORES):
    import concourse.mybir as _mybir
    ohnp = _mybir.dt.np(_mybir.dt.float8e4)

    WN = int(os.environ.get("GCN_W", "64"))   # dst window width
    wsh = WN.bit_length() - 1
    assert (1 << wsh) == WN

    x = np.asarray(x, dtype=np.float32)
    N, F = x.shape
    ns = N // n_cores
    assert ns * n_cores == N
    nblk = (ns + WN - 1) // WN
    npad = nblk * WN

    src = np.asarray(edge_index[0], dtype=np.int64)
    dst = np.asarray(edge_index[1], dtype=np.int64)
    deg = (np.bincount(dst, minlength=N) + 1).astype(np.float64)
    dinv = (1.0 / np.sqrt(deg)).astype(np.float32)
    xs = x * dinv[:, None]  # fold dinv[src] into all rows once

    # per-core edge lists (with self loops), sorted by dst block
    cores = []
    cnts = np.zeros((n_cores, nblk), dtype=np.int64)
    for c in range(n_cores):
        lo, hi = c * ns, (c + 1) * ns
        m = (dst >= lo) & (dst < hi)
        s_c = np.concatenate([src[m], np.arange(lo, hi)])
        d_c = np.concatenate([dst[m] - lo, np.arange(ns)])
        nd = dinv[np.concatenate([dst[m], np.arange(lo, hi)])]  # dinv[dst_e]
        blk = d_c >> wsh
        order = np.argsort(blk, kind="stable")
        s_c, d_c, nd, blk = s_c[order], d_c[order], nd[order], blk[order]
        cnts[c] = np.bincount(blk, minlength=nblk)
        cores.append((s_c, (d_c & (WN - 1)).astype(np.int64), nd, blk))

    # shared batch structure: per-block batch count = max over cores,
    # rounded up to even so every group's batch count is even (local_scatter
    # sub-calls must have an even index count)
    nbk = np.maximum(1, -(-cnts // 128)).max(axis=0)
    nbk = nbk + (nbk % 2)
    batch_base = np.concatenate([[0], np.cumsum(nbk)])

    # greedy group packing (shared across cores): small first group so the
    # PE starts early, tapered last groups so the PE backlog at DMA-end is
    # short (PE and DMA rates are nearly equal in steady state).
    Ball = int(batch_base[-1])
    groups = []
    k0 = 0
    while k0 < nblk:
        cnb = 0
        k1 = k0
        rem = Ball - int(batch_base[k0])
        if not groups:
            cap = max(int(nbk[0]), CNB_CAP // 4)
        elif rem > 2 * CNB_CAP:
            cap = CNB_CAP
        elif rem > 48:
            cap = max(int(nbk[k1]), 32)
        elif rem > 24:
            cap = max(int(nbk[k1]), 16)
        else:
            cap = max(int(nbk[k1]), 8)
        while k1 < nblk and cnb + nbk[k1] <= cap:
            cnb += int(nbk[k1])
            k1 += 1
        groups.append((k0, k1 - k0, int(batch_base[k0]), cnb))
        k0 = k1

    plan = Plan(F, WN, nblk, nbk, batch_base, groups, npad)

    mix = os.environ.get("GCN_MIX", "1,1")
    num, den = (int(v) for v in mix.split(","))
    plan.mix = (num, den)
    gen_flags = [(g % den) < num for g in range(len(groups))]
    plan.gen_flags = gen_flags
    plan.any_gen = any(gen_flags)
    plan.any_dma = not all(gen_flags)

    # per-block group-derived arrays for edge row addressing
    b0_of_blk = np.zeros(nblk, dtype=np.int64)
    cnb_of_blk = np.zeros(nblk, dtype=np.int64)
    for (k0, glen, b0, cnb) in groups:
        b0_of_blk[k0:k0 + glen] = b0
        cnb_of_blk[k0:k0 + glen] = cnb

    consts = {
        "wt": np.ascontiguousarray(np.asarray(W, np.float32).T).astype(
            np.float16),
        "bvec": np.asarray(b, np.float32).reshape(F, 1),
        "wreg": np.ascontiguousarray(
            np.asarray(w_reg, np.float32).T).astype(np.float16),
        "breg": np.asarray(b_reg, np.float32).reshape(1, 1),
    }

    Btot = plan.B
    for c in range(n_cores):
        s_c, rel_c, nd_c, blk_c = cores[c]
        bstart = np.searchsorted(blk_c, np.arange(nblk))
        q = np.arange(len(s_c)) - bstart[blk_c]       # pos within block
        ce = q >> 7                                    # batch within block
        pe = q & 127                                   # partition (slot)
        assert int((ce - nbk[blk_c] + 1).max()) <= 0, "block overflow"
        Be = batch_base[blk_c] + ce                    # global batch idx
        Re = (128 * b0_of_blk[blk_c] + pe * cnb_of_blk[blk_c]
              + (batch_base[blk_c] - b0_of_blk[blk_c]) + ce)

        xg = np.zeros((Btot * 128, F), dtype=np.float16)
        xg[Re] = (xs[s_c] * nd_c[:, None]).astype(np.float16)

        im = {"xg": xg, **consts}
        if plan.any_gen:
            # local_scatter indices: sub-calls cover 16 batches starting at
            # each group's first batch; the target column of slot p is
            # (offset-within-sub-call)*WN + rel. -1 (pad) is ignored by the
            # instruction.
            drl = np.full((128, Btot), -1, dtype=np.int16)
            drl[pe, Be] = ((Be - b0_of_blk[blk_c]) % 16) * WN + rel_c
            im["drl"] = drl
        if plan.any_dma:
            oh = np.zeros((128, Btot * WN), dtype=ohnp)
            oh[pe, Be * WN + rel_c] = 1.0
            im["oh"] = oh
        plan.in_maps.append(im)
    return plan


# ---------------------------------------------------------------------------
def build_nc(plan):
    import concourse.bacc as bacc
    import concourse.mybir as mybir
    import concourse.tile as tile

    f32 = mybir.dt.float32
    f16 = mybir.dt.float16
    oh8 = mybir.dt.float8e4
    F, WN, NBLK, NPAD, Btot = plan.F, plan.W, plan.NBLK, plan.NPAD, plan.B
    NBK, BB = plan.NBK, plan.BB
    BPB = CH // WN                      # W-blocks per psum bank

    dve_n = int(os.environ.get("GCN_DVE", "2"))
    pool_n = int(os.environ.get("GCN_POOL", "1"))

    nc = bacc.Bacc("TRN2", target_bir_lowering=False, debug=False)

    xg = nc.dram_tensor("xg", [Btot * 128, F], f16, kind="ExternalInput").ap()
    drl = (nc.dram_tensor("drl", [128, Btot], mybir.dt.int16,
                          kind="ExternalInput").ap()
           if plan.any_gen else None)
    oh = (nc.dram_tensor("oh", [128, Btot * WN], oh8,
                         kind="ExternalInput").ap()
          if plan.any_dma else None)
    wt = nc.dram_tensor("wt", [F, F], f16, kind="ExternalInput").ap()
    bvec = nc.dram_tensor("bvec", [F, 1], f32, kind="ExternalInput").ap()
    wreg = nc.dram_tensor("wreg", [F, 1], f16, kind="ExternalInput").ap()
    breg = nc.dram_tensor("breg", [1, 1], f32, kind="ExternalInput").ap()
    out = nc.dram_tensor("out", [1, NPAD], f32, kind="ExternalOutput").ap()

    with tile.TileContext(nc) as tc:
        with (
            tc.tile_pool(name="const", bufs=1) as cpool,
            tc.tile_pool(name="stream", bufs=6) as spool,
            tc.tile_pool(name="ohp", bufs=8) as ogpool,
            tc.tile_pool(name="ps", bufs=5, space="PSUM") as pspool,
            tc.tile_pool(name="ph2", bufs=2, space="PSUM") as ph2pool,
            tc.tile_pool(name="po", bufs=1, space="PSUM") as popool,
            tc.tile_pool(name="hrelu", bufs=2) as hpool,
        ):
            wt_sb = cpool.tile([F, F], f16)
            b_sb = cpool.tile([F, 1], f32)
            wreg_sb = cpool.tile([F, 1], f16)
            breg_sb = cpool.tile([1, 1], f32)
            accT = cpool.tile([128, NPAD], f16)
            out_sb = cpool.tile([1, NPAD], f32)

            for sb, dr in ((wt_sb, wt), (b_sb, bvec), (wreg_sb, wreg),
                           (breg_sb, breg)):
                nc.scalar.dma_start(out=sb[:], in_=dr[:])

            if plan.any_gen:
                drl_sb = cpool.tile([128, Btot], mybir.dt.int16)
                nc.sync.dma_start(out=drl_sb[:], in_=drl[:])
                ones_sb = cpool.tile([128, 16], f16)
                nc.vector.memset(ones_sb[:], 1.0)

            # phase-2 two-stage pipeline, decoupled from the scatter stream:
            # stage A (W^T @ accT chunk) fires one bank after the chunk's
            # CAST; stage B (relu result -> cox row) one bank after stage A —
            # so each PE-queue entry's dependency is already satisfied when
            # it reaches the head (the in-order PE queue never stalls on
            # DVE/ACT work).
            pend_a = []  # chunks awaiting stage A
            pend_b = []  # (c0, c1, hr) awaiting stage B

            def phase2_a(c0, c1):
                cw = c1 - c0
                ph = ph2pool.tile([128, CH], f32)
                hr = hpool.tile([128, CH], f16)
                nc.tensor.matmul(ph[:, :cw], lhsT=wt_sb[:],
                                 rhs=accT[:, c0:c1], start=True, stop=True)
                nc.scalar.activation(hr[:, :cw], ph[:, :cw],
                                     mybir.ActivationFunctionType.Relu,
                                     bias=b_sb[:, :1])
                return hr

            out_done = [0]

            def phase2_b(c0, c1, hr):
                cw = c1 - c0
                po = popool.tile([1, CH], f32)
                nc.tensor.matmul(po[:, :cw], lhsT=wreg_sb[:], rhs=hr[:, :cw],
                                 start=True, stop=True)
                nc.scalar.activation(out_sb[:, c0:c1], po[:, :cw],
                                     mybir.ActivationFunctionType.Identity,
                                     bias=breg_sb[:, :1])
                # stream the finished half of the output row early so the
                # final out-DMA is small
                if out_done[0] == 0 and c1 >= NPAD // 2:
                    nc.scalar.dma_start(out=out[:, :c1], in_=out_sb[:, :c1])
                    out_done[0] = c1

            def pump_phase2(final=False):
                while pend_b and (final or len(pend_b) > 1):
                    phase2_b(*pend_b.pop(0))
                    if final:  # drain: issue remaining stage As first so
                        break  # they pipeline ahead of the B chain
                while pend_a and (final or len(pend_a) > 1):
                    c0, c1 = pend_a.pop(0)
                    pend_b.append((c0, c1, phase2_a(c0, c1)))
                while pend_b and final:
                    phase2_b(*pend_b.pop(0))

            done_cols = 0
            ps = None
            for gi, (k0, glen, b0, cnb) in enumerate(plan.GROUPS):
                gen = plan.gen_flags[gi]
                st = spool.tile([128, CNB_CAP * F], f16, tag="st")
                q = nc.sync
                q.dma_start(
                    out=st[:, :cnb * F].rearrange("p (c f) -> p c f", f=F),
                    in_=xg[128 * b0:128 * (b0 + cnb), :].rearrange(
                        "(p c) f -> p c f", p=128),
                )
                if gen:
                    ot = ogpool.tile([128, CNB_CAP * WN], f16, tag="ot")
                    j0 = 0
                    while j0 < cnb:
                        sub = min(16, cnb - j0)
                        nc.gpsimd.local_scatter(
                            out_ap=ot[:, j0 * WN:(j0 + sub) * WN],
                            data_ap=ones_sb[:, :sub],
                            idxs_ap=drl_sb[:, b0 + j0:b0 + j0 + sub],
                            channels=128,
                            num_elems=sub * WN,
                            num_idxs=sub,
                        )
                        j0 += sub
                else:
                    ot = ogpool.tile([128, CNB_CAP * WN], oh8, tag="ot")
                    q2 = nc.sync
                    q2.dma_start(out=ot[:, :cnb * WN],
                                 in_=oh[:, b0 * WN:(b0 + cnb) * WN])

                for i in range(glen):
                    k = k0 + i
                    nb = int(NBK[k])
                    loff = int(BB[k]) - b0
                    kb = k % BPB        # window within psum bank
                    if kb == 0:
                        ps = pspool.tile([128, CH], f32)
                    for cc in range(nb):
                        j = loff + cc
                        nc.tensor.matmul(ps[:, kb * WN:(kb + 1) * WN],
                                         lhsT=st[:, j * F:(j + 1) * F],
                                         rhs=ot[:, j * WN:(j + 1) * WN],
                                         start=(cc == 0), stop=(cc == nb - 1))
                    if kb == BPB - 1 or k == NBLK - 1:
                        cb0 = (k - kb) * WN
                        cw = (kb + 1) * WN
                        nc.vector.tensor_copy(accT[:, cb0:cb0 + cw],
                                              ps[:, :cw])
                        avail = (k + 1) * WN
                        while done_cols + CH <= avail or (k == NBLK - 1
                                                         and done_cols < NPAD):
                            c1 = min(done_cols + CH, NPAD)
                            if c1 == NPAD and c1 - done_cols > CH // 2:
                                # split the last chunk: halves the length of
                                # the final serial phase-2 dependency chain
                                mid = done_cols + (c1 - done_cols) // 2
                                pend_a.append((done_cols, mid))
                                pend_a.append((mid, c1))
                            else:
                                pend_a.append((done_cols, c1))
                            done_cols = c1
                        pump_phase2(final=(k == NBLK - 1))

            nc.sync.dma_start(out=out[:, out_done[0]:],
                              in_=out_sb[:, out_done[0]:])

    nc.compile()
    return nc


# ---------------------------------------------------------------------------
_CACHE = {}


def _ensure_ntff_hook():
    try:
        from antenv.axon_hooks import get_axon_ntff_profile_hook  # noqa: F401
        return
    except ImportError:
        pass
    import sys
    import types
    import antenv
    mod = types.ModuleType("antenv.axon_hooks")
    mod._hook = None
    mod.set_axon_ntff_profile_hook = lambda h: setattr(mod, "_hook", h)
    mod.get_axon_ntff_profile_hook = lambda: mod._hook
    sys.modules["antenv.axon_hooks"] = mod
    antenv.axon_hooks = mod
    try:
        from trn_agent_boot.trn_boot import _ntff_profile_via_ctypes
        mod._hook = _ntff_profile_via_ctypes("/opt/axon/libaxon_pjrt.so")
    except Exception:
        pass


def _run(plan, nc, trace=False):
    import concourse.bass_utils as bu
    if trace:
        _ensure_ntff_hook()
        bu.upload_artifacts = lambda tmpdir: tmpdir  # no egress here
    core_ids = list(range(len(plan.in_maps)))
    res = bu.run_bass_kernel_spmd(nc, plan.in_maps, core_ids, trace=trace)
    return res


def kernel(x, edge_index, W, b, w_reg, b_reg):
    trace = bool(os.environ.get("GCN_TRACE"))

    plan = make_plan(x, edge_index, W, b, w_reg, b_reg)
    key = (plan.B, plan.W, CNB_CAP, tuple(plan.NBK.tolist()), plan.mix,
           os.environ.get("GCN_DVE"), os.environ.get("GCN_POOL"))
    if key not in _CACHE:
        _CACHE[key] = build_nc(plan)
    nc = _CACHE[key]

    res = None
    for attempt in range(3):
        try:
            res = _run(plan, nc, trace=trace)
            break
        except Exception:
            # transient device errors (e.g. NRT exec-unit resets) recover on
            # a fresh attempt; re-raise only if persistent
            if attempt == 2:
                raise
            time.sleep(5.0)
    kernel.last_exec_ns = res.exec_time_ns
    kernel.last_profile = res.profile_json

    N = np.asarray(x).shape[0]
    ns = N // len(plan.in_maps)
    shards = [res.results[c]["out"][0, :ns] for c in range(len(plan.in_maps))]
    return np.concatenate(shards).reshape(N, 1).astype(np.float32)


kernel.last_exec_ns = None
kernel.last_profile = None


# revision 32
# speedup vs baseline: 1.0740x; 1.0106x over previous
"""GCN (single GCNConv + Cox head) Trainium2 Bass kernel, 8-core SPMD.

Math (per reference):
    src,dst += self loops;  deg = indegree(dst);  dinv = deg^-1/2
    agg[d]  = sum_e 1[dst_e = d] * (dinv[src_e] * dinv[dst_e] * x[src_e])
    out     = relu(agg @ W.T + b) @ w_reg.T + b_reg

Distribution: destination-sharded over 8 cores (12500 dst nodes each), no
collectives — each core gets host-staged per-edge row tables and writes its
output shard; the host concatenates shards.

Design: every edge slot is streamed by HWDGE on the SP queue only (no SWDGE
gather — SWDGE descriptor processing stalls the shared DMA engines at
~2 GB/s; and no DMA issues on the ACT queue — a blocked ACTIVATE would
delay stream issues queued behind it). The full per-edge norm
dinv[src]*dinv[dst] is folded into the streamed fp16 row on the host, so
the scatter one-hot is exactly 0/1 and psum comes out pre-scaled. Matmul
orientation puts features on psum partitions:
    psum[F, dstW] += msg[slot, F].T @ onehot[slot, dstW]
so accT is feat-major with a plain chunked copy (no PE transpose, no ACT
scale). dst windows are W=64 wide (env GCN_W): halves one-hot size and
matmul moving time vs 128. Eight W-windows accumulate into one psum bank
[128, 512]; a single DVE cast per bank lands them in accT. Phase 2
(W^T @ accT chunk, relu+bias, cox row) runs as a two-stage pipeline one
bank behind the scatter stream so no PE-queue entry waits on DVE/ACT work.

The one-hot is generated ON-CHIP by gpsimd local_scatter (dst[:]=0;
dst[:,idx]=1 per partition, 16 batches per call) from a [128, B] int16
index table — ~55us on the otherwise-idle Pool engine instead of 6.4 MB
of fp8 DMA. (tensor_scalar is_equal was tried first: ~2.7us/batch custom
DVE ucode — dead end. GCN_MIX=0,1 falls back to DMA'd fp8 one-hots.)
Remaining HBM traffic is ~25.9 MB/core, sustained at ~425 GB/s.
"""

import os
import time
import numpy as np

N_CORES = 8
CNB_CAP = int(os.environ.get("GCN_CNB", "48"))  # batches per stream-DMA group
CH = 512          # phase-2 column chunk == psum bank width


class Plan:
    def __init__(self, n_feat, w, nblk, nbk, batch_base, groups, npad):
        self.F = n_feat
        self.W = w
        self.NBLK = nblk
        self.NBK = nbk                  # [nblk] batches per block
        self.BB = batch_base            # [nblk+1] cumsum of NBK
        self.GROUPS = groups            # (k0, glen, b0, cnb)
        self.B = int(batch_base[-1])    # total batches per core
        self.NPAD = npad
        self.in_maps = []
        self.mix = (0, 1)
        self.any_gen = False
        self.any_dma = True


def make_plan(x, edge_index, W, b, w_reg, b_reg, n_cores=N_CORES):
    import concourse.mybir as _mybir
    ohnp = _mybir.dt.np(_mybir.dt.float8e4)

    WN = int(os.environ.get("GCN_W", "64"))   # dst window width
    wsh = WN.bit_length() - 1
    assert (1 << wsh) == WN

    x = np.asarray(x, dtype=np.float32)
    N, F = x.shape
    ns = N // n_cores
    assert ns * n_cores == N
    nblk = (ns + WN - 1) // WN
    npad = nblk * WN

    src = np.asarray(edge_index[0], dtype=np.int64)
    dst = np.asarray(edge_index[1], dtype=np.int64)
    deg = (np.bincount(dst, minlength=N) + 1).astype(np.float64)
    dinv = (1.0 / np.sqrt(deg)).astype(np.float32)
    xs = x * dinv[:, None]  # fold dinv[src] into all rows once

    # per-core edge lists (with self loops), sorted by dst block
    cores = []
    cnts = np.zeros((n_cores, nblk), dtype=np.int64)
    for c in range(n_cores):
        lo, hi = c * ns, (c + 1) * ns
        m = (dst >= lo) & (dst < hi)
        s_c = np.concatenate([src[m], np.arange(lo, hi)])
        d_c = np.concatenate([dst[m] - lo, np.arange(ns)])
        nd = dinv[np.concatenate([dst[m], np.arange(lo, hi)])]  # dinv[dst_e]
        blk = d_c >> wsh
        order = np.argsort(blk, kind="stable")
        s_c, d_c, nd, blk = s_c[order], d_c[order], nd[order], blk[order]
        cnts[c] = np.bincount(blk, minlength=nblk)
        cores.append((s_c, (d_c & (WN - 1)).astype(np.int64), nd, blk))

    # shared batch structure: per-block batch count = max over cores,
    # rounded up to even so every group's batch count is even (local_scatter
    # sub-calls must have an even index count)
    nbk = np.maximum(1, -(-cnts // 128)).max(axis=0)
    nbk = nbk + (nbk % 2)
    batch_base = np.concatenate([[0], np.cumsum(nbk)])

    # greedy group packing (shared across cores): small first group so the
    # PE starts early, tapered last groups so the PE backlog at DMA-end is
    # short (PE and DMA rates are nearly equal in steady state).
    Ball = int(batch_base[-1])
    groups = []
    k0 = 0
    while k0 < nblk:
        cnb = 0
        k1 = k0
        rem = Ball - int(batch_base[k0])
        if not groups:
            cap = max(int(nbk[0]), CNB_CAP // 4)
        elif rem > 2 * CNB_CAP:
            cap = CNB_CAP
        elif rem > 48:
            cap = max(int(nbk[k1]), 32)
        elif rem > 24:
            cap = max(int(nbk[k1]), 16)
        else:
            cap = max(int(nbk[k1]), 8)
        while k1 < nblk and cnb + nbk[k1] <= cap:
            cnb += int(nbk[k1])
            k1 += 1
        groups.append((k0, k1 - k0, int(batch_base[k0]), cnb))
        k0 = k1

    plan = Plan(F, WN, nblk, nbk, batch_base, groups, npad)

    mix = os.environ.get("GCN_MIX", "1,1")
    num, den = (int(v) for v in mix.split(","))
    plan.mix = (num, den)
    gen_flags = [(g % den) < num for g in range(len(groups))]
    plan.gen_flags = gen_flags
    plan.any_gen = any(gen_flags)
    plan.any_dma = not all(gen_flags)

    # per-block group-derived arrays for edge row addressing
    b0_of_blk = np.zeros(nblk, dtype=np.int64)
    cnb_of_blk = np.zeros(nblk, dtype=np.int64)
    for (k0, glen, b0, cnb) in groups:
        b0_of_blk[k0:k0 + glen] = b0
        cnb_of_blk[k0:k0 + glen] = cnb

    consts = {
        "wt": np.ascontiguousarray(np.asarray(W, np.float32).T).astype(
            np.float16),
        "bvec": np.asarray(b, np.float32).reshape(F, 1),
        "wreg": np.ascontiguousarray(
            np.asarray(w_reg, np.float32).T).astype(np.float16),
        "breg": np.asarray(b_reg, np.float32).reshape(1, 1),
    }

    Btot = plan.B
    for c in range(n_cores):
        s_c, rel_c, nd_c, blk_c = cores[c]
        bstart = np.searchsorted(blk_c, np.arange(nblk))
        q = np.arange(len(s_c)) - bstart[blk_c]       # pos within block
        ce = q >> 7                                    # batch within block
        pe = q & 127                                   # partition (slot)
        assert int((ce - nbk[blk_c] + 1).max()) <= 0, "block overflow"
        Be = batch_base[blk_c] + ce                    # global batch idx
        Re = (128 * b0_of_blk[blk_c] + pe * cnb_of_blk[blk_c]
              + (batch_base[blk_c] - b0_of_blk[blk_c]) + ce)

        xg = np.zeros((Btot * 128, F), dtype=np.float16)
        xg[Re] = (xs[s_c] * nd_c[:, None]).astype(np.float16)

        im = {"xg": xg, **consts}
        if plan.any_gen:
            # local_scatter indices: sub-calls cover 16 batches starting at
            # each group's first batch; the target column of slot p is
            # (offset-within-sub-call)*WN + rel. -1 (pad) is ignored by the
            # instruction.
            drl = np.full((128, Btot), -1, dtype=np.int16)
            drl[pe, Be] = ((Be - b0_of_blk[blk_c]) % 16) * WN + rel_c
            im["drl"] = drl
        if plan.any_dma:
            oh = np.zeros((128, Btot * WN), dtype=ohnp)
            oh[pe, Be * WN + rel_c] = 1.0
            im["oh"] = oh
        plan.in_maps.append(im)
    return plan


# ---------------------------------------------------------------------------
def build_nc(plan):
    import concourse.bacc as bacc
    import concourse.mybir as mybir
    import concourse.tile as tile

    f32 = mybir.dt.float32
    f16 = mybir.dt.float16
    oh8 = mybir.dt.float8e4
    F, WN, NBLK, NPAD, Btot = plan.F, plan.W, plan.NBLK, plan.NPAD, plan.B
    NBK, BB = plan.NBK, plan.BB
    BPB = CH // WN                      # W-blocks per psum bank

    dve_n = int(os.environ.get("GCN_DVE", "2"))
    pool_n = int(os.environ.get("GCN_POOL", "1"))

    nc = bacc.Bacc("TRN2", target_bir_lowering=False, debug=False)

    xg = nc.dram_tensor("xg", [Btot * 128, F], f16, kind="ExternalInput").ap()
    drl = (nc.dram_tensor("drl", [128, Btot], mybir.dt.int16,
                          kind="ExternalInput").ap()
           if plan.any_gen else None)
    oh = (nc.dram_tensor("oh", [128, Btot * WN], oh8,
                         kind="ExternalInput").ap()
          if plan.any_dma else None)
    wt = nc.dram_tensor("wt", [F, F], f16, kind="ExternalInput").ap()
    bvec = nc.dram_tensor("bvec", [F, 1], f32, kind="ExternalInput").ap()
    wreg = nc.dram_tensor("wreg", [F, 1], f16, kind="ExternalInput").ap()
    breg = nc.dram_tensor("breg", [1, 1], f32, kind="ExternalInput").ap()
    out = nc.dram_tensor("out", [1, NPAD], f32, kind="ExternalOutput").ap()

    with tile.TileContext(nc) as tc:
        with (
            tc.tile_pool(name="const", bufs=1) as cpool,
            tc.tile_pool(name="stream", bufs=5) as spool,
            tc.tile_pool(name="ohp", bufs=6) as ogpool,
            tc.tile_pool(name="ps", bufs=5, space="PSUM") as pspool,
            tc.tile_pool(name="ph2", bufs=2, space="PSUM") as ph2pool,
            tc.tile_pool(name="po", bufs=1, space="PSUM") as popool,
            tc.tile_pool(name="hrelu", bufs=2) as hpool,
        ):
            wt_sb = cpool.tile([F, F], f16)
            b_sb = cpool.tile([F, 1], f32)
            wreg_sb = cpool.tile([F, 1], f16)
            breg_sb = cpool.tile([1, 1], f32)
            accT = cpool.tile([128, NPAD], f16)
            out_sb = cpool.tile([1, NPAD], f32)

            for sb, dr in ((wt_sb, wt), (b_sb, bvec), (wreg_sb, wreg),
                           (breg_sb, breg)):
                nc.scalar.dma_start(out=sb[:], in_=dr[:])

            if plan.any_gen:
                drl_sb = cpool.tile([128, Btot], mybir.dt.int16)
                nc.sync.dma_start(out=drl_sb[:], in_=drl[:])
                ones_sb = cpool.tile([128, 16], f16)
                nc.vector.memset(ones_sb[:], 1.0)

            # phase-2 two-stage pipeline, decoupled from the scatter stream:
            # stage A (W^T @ accT chunk) fires one bank after the chunk's
            # CAST; stage B (relu result -> cox row) one bank after stage A —
            # so each PE-queue entry's dependency is already satisfied when
            # it reaches the head (the in-order PE queue never stalls on
            # DVE/ACT work).
            pend_a = []  # chunks awaiting stage A
            pend_b = []  # (c0, c1, hr) awaiting stage B

            def phase2_a(c0, c1):
                cw = c1 - c0
                ph = ph2pool.tile([128, CH], f32)
                hr = hpool.tile([128, CH], f16)
                nc.tensor.matmul(ph[:, :cw], lhsT=wt_sb[:],
                                 rhs=accT[:, c0:c1], start=True, stop=True)
                nc.scalar.activation(hr[:, :cw], ph[:, :cw],
                                     mybir.ActivationFunctionType.Relu,
                                     bias=b_sb[:, :1])
                return hr

            out_done = [0]

            def phase2_b(c0, c1, hr):
                cw = c1 - c0
                po = popool.tile([1, CH], f32)
                nc.tensor.matmul(po[:, :cw], lhsT=wreg_sb[:], rhs=hr[:, :cw],
                                 start=True, stop=True)
                nc.scalar.activation(out_sb[:, c0:c1], po[:, :cw],
                                     mybir.ActivationFunctionType.Identity,
                                     bias=breg_sb[:, :1])
                # stream the finished half of the output row early so the
                # final out-DMA is small
                if out_done[0] == 0 and c1 >= NPAD // 2:
                    nc.scalar.dma_start(out=out[:, :c1], in_=out_sb[:, :c1])
                    out_done[0] = c1

            def pump_phase2(final=False):
                while pend_b and (final or len(pend_b) > 1):
                    phase2_b(*pend_b.pop(0))
                    if final:  # drain: issue remaining stage As first so
                        break  # they pipeline ahead of the B chain
                while pend_a and (final or len(pend_a) > 1):
                    c0, c1 = pend_a.pop(0)
                    pend_b.append((c0, c1, phase2_a(c0, c1)))
                while pend_b and final:
                    phase2_b(*pend_b.pop(0))

            done_cols = 0
            ps = None
            for gi, (k0, glen, b0, cnb) in enumerate(plan.GROUPS):
                gen = plan.gen_flags[gi]
                st = spool.tile([128, CNB_CAP * F], f16, tag="st")
                q = nc.sync
                q.dma_start(
                    out=st[:, :cnb * F].rearrange("p (c f) -> p c f", f=F),
                    in_=xg[128 * b0:128 * (b0 + cnb), :].rearrange(
                        "(p c) f -> p c f", p=128),
                )
                if gen:
                    ot = ogpool.tile([128, CNB_CAP * WN], f16, tag="ot")
                    j0 = 0
                    while j0 < cnb:
                        sub = min(16, cnb - j0)
                        nc.gpsimd.local_scatter(
                            out_ap=ot[:, j0 * WN:(j0 + sub) * WN],
                            data_ap=ones_sb[:, :sub],
                            idxs_ap=drl_sb[:, b0 + j0:b0 + j0 + sub],
                            channels=128,
                            num_elems=sub * WN,
                            num_idxs=sub,
                        )
                        j0 += sub
                else:
                    ot = ogpool.tile([128, CNB_CAP * WN], oh8, tag="ot")
                    q2 = nc.sync
                    q2.dma_start(out=ot[:, :cnb * WN],
                                 in_=oh[:, b0 * WN:(b0 + cnb) * WN])

                for i in range(glen):
                    k = k0 + i
                    nb = int(NBK[k])
                    loff = int(BB[k]) - b0
                    kb = k % BPB        # window within psum bank
                    if kb == 0:
                        ps = pspool.tile([128, CH], f32)
                    for cc in range(nb):
                        j = loff + cc
                        nc.tensor.matmul(ps[:, kb * WN:(kb + 1) * WN],
                                         lhsT=st[:, j * F:(j + 1) * F],
                                         rhs=ot[:, j * WN:(j + 1) * WN],
                                         start=(cc == 0), stop=(cc == nb - 1))
                    if kb == BPB - 1 or k == NBLK - 1:
                        cb0 = (k - kb) * WN
                        cw = (kb + 1) * WN
                        nc.vector.tensor_copy(accT[:, cb0:cb0 + cw],
                                              ps[:, :cw])
                        avail = (k + 1) * WN
                        while done_cols + CH <= avail or (k == NBLK - 1
                                                         and done_cols < NPAD):
                            c1 = min(done_cols + CH, NPAD)
                            if c1 == NPAD and c1 - done_cols > CH // 2:
                                # split the last chunk: halves the length of
                                # the final serial phase-2 dependency chain
                                mid = done_cols + (c1 - done_cols) // 2
                                pend_a.append((done_cols, mid))
                                pend_a.append((mid, c1))
                            else:
                                pend_a.append((done_cols, c1))
                            done_cols = c1
                        pump_phase2(final=(k == NBLK - 1))

            nc.sync.dma_start(out=out[:, out_done[0]:],
                              in_=out_sb[:, out_done[0]:])

    nc.compile()
    return nc


# ---------------------------------------------------------------------------
_CACHE = {}


def _ensure_ntff_hook():
    try:
        from antenv.axon_hooks import get_axon_ntff_profile_hook  # noqa: F401
        return
    except ImportError:
        pass
    import sys
    import types
    import antenv
    mod = types.ModuleType("antenv.axon_hooks")
    mod._hook = None
    mod.set_axon_ntff_profile_hook = lambda h: setattr(mod, "_hook", h)
    mod.get_axon_ntff_profile_hook = lambda: mod._hook
    sys.modules["antenv.axon_hooks"] = mod
    antenv.axon_hooks = mod
    try:
        from trn_agent_boot.trn_boot import _ntff_profile_via_ctypes
        mod._hook = _ntff_profile_via_ctypes("/opt/axon/libaxon_pjrt.so")
    except Exception:
        pass


def _run(plan, nc, trace=False):
    import concourse.bass_utils as bu
    if trace:
        _ensure_ntff_hook()
        bu.upload_artifacts = lambda tmpdir: tmpdir  # no egress here
    core_ids = list(range(len(plan.in_maps)))
    res = bu.run_bass_kernel_spmd(nc, plan.in_maps, core_ids, trace=trace)
    return res


def kernel(x, edge_index, W, b, w_reg, b_reg):
    trace = bool(os.environ.get("GCN_TRACE"))

    plan = make_plan(x, edge_index, W, b, w_reg, b_reg)
    key = (plan.B, plan.W, CNB_CAP, tuple(plan.NBK.tolist()), plan.mix,
           os.environ.get("GCN_DVE"), os.environ.get("GCN_POOL"))
    if key not in _CACHE:
        _CACHE[key] = build_nc(plan)
    nc = _CACHE[key]

    res = None
    for attempt in range(3):
        try:
            res = _run(plan, nc, trace=trace)
            break
        except Exception:
            # transient device errors (e.g. NRT exec-unit resets) recover on
            # a fresh attempt; re-raise only if persistent
            if attempt == 2:
                raise
            time.sleep(5.0)
    kernel.last_exec_ns = res.exec_time_ns
    kernel.last_profile = res.profile_json

    N = np.asarray(x).shape[0]
    ns = N // len(plan.in_maps)
    shards = [res.results[c]["out"][0, :ns] for c in range(len(plan.in_maps))]
    return np.concatenate(shards).reshape(N, 1).astype(np.float32)


kernel.last_exec_ns = None
kernel.last_profile = None


# revision 33
# speedup vs baseline: 1.1131x; 1.0363x over previous
"""GCN (single GCNConv + Cox head) Trainium2 Bass kernel, 8-core SPMD.

Math (per reference):
    src,dst += self loops;  deg = indegree(dst);  dinv = deg^-1/2
    agg[d]  = sum_e 1[dst_e = d] * (dinv[src_e] * dinv[dst_e] * x[src_e])
    out     = relu(agg @ W.T + b) @ w_reg.T + b_reg

Distribution: destination-sharded over 8 cores (12500 dst nodes each), no
collectives — each core gets host-staged per-edge row tables and writes its
output shard; the host concatenates shards.

Design: every edge slot is streamed by HWDGE on the SP queue only (no SWDGE
gather — SWDGE descriptor processing stalls the shared DMA engines at
~2 GB/s; and no DMA issues on the ACT queue — a blocked ACTIVATE would
delay stream issues queued behind it). The full per-edge norm
dinv[src]*dinv[dst] is folded into the streamed fp16 row on the host, so
the scatter one-hot is exactly 0/1 and psum comes out pre-scaled. Matmul
orientation puts features on psum partitions:
    psum[F, dstW] += msg[slot, F].T @ onehot[slot, dstW]
so accT is feat-major with a plain chunked copy (no PE transpose, no ACT
scale). dst windows are W=64 wide (env GCN_W): halves one-hot size and
matmul moving time vs 128. Eight W-windows accumulate into one psum bank
[128, 512]; a single DVE cast per bank lands them in accT. Phase 2
(W^T @ accT chunk, relu+bias, cox row) runs as a two-stage pipeline one
bank behind the scatter stream so no PE-queue entry waits on DVE/ACT work.

The one-hot is generated ON-CHIP by gpsimd local_scatter (dst[:]=0;
dst[:,idx]=1 per partition, 16 batches per call) from a [128, B] int16
index table — ~55us on the otherwise-idle Pool engine instead of 6.4 MB
of fp8 DMA. (tensor_scalar is_equal was tried first: ~2.7us/batch custom
DVE ucode — dead end. GCN_MIX=0,1 falls back to DMA'd fp8 one-hots.)
Remaining HBM traffic is ~25.9 MB/core, sustained at ~425 GB/s.
"""

import os
import time
import numpy as np

N_CORES = 8
CNB_CAP = int(os.environ.get("GCN_CNB", "48"))  # batches per stream-DMA group
CH = 512          # phase-2 column chunk == psum bank width


class Plan:
    def __init__(self, n_feat, w, nblk, nbk, batch_base, groups, npad):
        self.F = n_feat
        self.W = w
        self.NBLK = nblk
        self.NBK = nbk                  # [nblk] batches per block
        self.BB = batch_base            # [nblk+1] cumsum of NBK
        self.GROUPS = groups            # (k0, glen, b0, cnb)
        self.B = int(batch_base[-1])    # total batches per core
        self.NPAD = npad
        self.in_maps = []
        self.mix = (0, 1)
        self.any_gen = False
        self.any_dma = True


def make_plan(x, edge_index, W, b, w_reg, b_reg, n_cores=N_CORES):
    import concourse.mybir as _mybir
    ohnp = _mybir.dt.np(_mybir.dt.float8e4)

    WN = int(os.environ.get("GCN_W", "64"))   # dst window width
    wsh = WN.bit_length() - 1
    assert (1 << wsh) == WN

    x = np.asarray(x, dtype=np.float32)
    N, F = x.shape
    ns = N // n_cores
    assert ns * n_cores == N
    nblk = (ns + WN - 1) // WN
    npad = nblk * WN

    src = np.asarray(edge_index[0], dtype=np.int64)
    dst = np.asarray(edge_index[1], dtype=np.int64)
    deg = (np.bincount(dst, minlength=N) + 1).astype(np.float64)
    dinv = (1.0 / np.sqrt(deg)).astype(np.float32)
    xs = x * dinv[:, None]  # fold dinv[src] into all rows once

    # per-core edge lists (with self loops), sorted by dst block
    cores = []
    cnts = np.zeros((n_cores, nblk), dtype=np.int64)
    for c in range(n_cores):
        lo, hi = c * ns, (c + 1) * ns
        m = (dst >= lo) & (dst < hi)
        s_c = np.concatenate([src[m], np.arange(lo, hi)])
        d_c = np.concatenate([dst[m] - lo, np.arange(ns)])
        nd = dinv[np.concatenate([dst[m], np.arange(lo, hi)])]  # dinv[dst_e]
        blk = d_c >> wsh
        order = np.argsort(blk, kind="stable")
        s_c, d_c, nd, blk = s_c[order], d_c[order], nd[order], blk[order]
        cnts[c] = np.bincount(blk, minlength=nblk)
        cores.append((s_c, (d_c & (WN - 1)).astype(np.int64), nd, blk))

    # shared batch structure: per-block batch count = max over cores,
    # rounded up to even so every group's batch count is even (local_scatter
    # sub-calls must have an even index count)
    nbk = np.maximum(1, -(-cnts // 128)).max(axis=0)
    nbk = nbk + (nbk % 2)
    batch_base = np.concatenate([[0], np.cumsum(nbk)])

    # greedy group packing (shared across cores): small first group so the
    # PE starts early, tapered last groups so the PE backlog at DMA-end is
    # short (PE and DMA rates are nearly equal in steady state).
    Ball = int(batch_base[-1])
    groups = []
    k0 = 0
    while k0 < nblk:
        cnb = 0
        k1 = k0
        rem = Ball - int(batch_base[k0])
        if not groups:
            cap = max(int(nbk[0]), CNB_CAP // 4)
        elif rem > 2 * CNB_CAP:
            cap = CNB_CAP
        elif rem > 48:
            cap = max(int(nbk[k1]), 32)
        elif rem > 24:
            cap = max(int(nbk[k1]), 16)
        else:
            cap = max(int(nbk[k1]), 8)
        while k1 < nblk and cnb + nbk[k1] <= cap:
            cnb += int(nbk[k1])
            k1 += 1
        groups.append((k0, k1 - k0, int(batch_base[k0]), cnb))
        k0 = k1

    plan = Plan(F, WN, nblk, nbk, batch_base, groups, npad)

    mix = os.environ.get("GCN_MIX", "1,1")
    num, den = (int(v) for v in mix.split(","))
    plan.mix = (num, den)
    gen_flags = [(g % den) < num for g in range(len(groups))]
    plan.gen_flags = gen_flags
    plan.any_gen = any(gen_flags)
    plan.any_dma = not all(gen_flags)

    # per-block group-derived arrays for edge row addressing
    b0_of_blk = np.zeros(nblk, dtype=np.int64)
    cnb_of_blk = np.zeros(nblk, dtype=np.int64)
    for (k0, glen, b0, cnb) in groups:
        b0_of_blk[k0:k0 + glen] = b0
        cnb_of_blk[k0:k0 + glen] = cnb

    consts = {
        "wt": np.ascontiguousarray(np.asarray(W, np.float32).T).astype(
            np.float16),
        "bvec": np.asarray(b, np.float32).reshape(F, 1),
        "wreg": np.ascontiguousarray(
            np.asarray(w_reg, np.float32).T).astype(np.float16),
        "breg": np.asarray(b_reg, np.float32).reshape(1, 1),
    }

    Btot = plan.B
    for c in range(n_cores):
        s_c, rel_c, nd_c, blk_c = cores[c]
        bstart = np.searchsorted(blk_c, np.arange(nblk))
        q = np.arange(len(s_c)) - bstart[blk_c]       # pos within block
        ce = q >> 7                                    # batch within block
        pe = q & 127                                   # partition (slot)
        assert int((ce - nbk[blk_c] + 1).max()) <= 0, "block overflow"
        Be = batch_base[blk_c] + ce                    # global batch idx
        Re = (128 * b0_of_blk[blk_c] + pe * cnb_of_blk[blk_c]
              + (batch_base[blk_c] - b0_of_blk[blk_c]) + ce)

        xg = np.zeros((Btot * 128, F), dtype=np.float16)
        xg[Re] = (xs[s_c] * nd_c[:, None]).astype(np.float16)

        im = {"xg": xg, **consts}
        if plan.any_gen:
            # local_scatter indices: sub-calls cover 16 batches starting at
            # each group's first batch; the target column of slot p is
            # (offset-within-sub-call)*WN + rel. -1 (pad) is ignored by the
            # instruction.
            drl = np.full((128, Btot), -1, dtype=np.int16)
            drl[pe, Be] = ((Be - b0_of_blk[blk_c]) % 16) * WN + rel_c
            im["drl"] = drl
        if plan.any_dma:
            oh = np.zeros((128, Btot * WN), dtype=ohnp)
            oh[pe, Be * WN + rel_c] = 1.0
            im["oh"] = oh
        plan.in_maps.append(im)
    return plan


# ---------------------------------------------------------------------------
def build_nc(plan):
    import concourse.bacc as bacc
    import concourse.mybir as mybir
    import concourse.tile as tile

    f32 = mybir.dt.float32
    f16 = mybir.dt.float16
    oh8 = mybir.dt.float8e4
    F, WN, NBLK, NPAD, Btot = plan.F, plan.W, plan.NBLK, plan.NPAD, plan.B
    NBK, BB = plan.NBK, plan.BB
    BPB = CH // WN                      # W-blocks per psum bank

    dve_n = int(os.environ.get("GCN_DVE", "2"))
    pool_n = int(os.environ.get("GCN_POOL", "1"))

    nc = bacc.Bacc("TRN2", target_bir_lowering=False, debug=False)

    xg = nc.dram_tensor("xg", [Btot * 128, F], f16, kind="ExternalInput").ap()
    drl = (nc.dram_tensor("drl", [128, Btot], mybir.dt.int16,
                          kind="ExternalInput").ap()
           if plan.any_gen else None)
    oh = (nc.dram_tensor("oh", [128, Btot * WN], oh8,
                         kind="ExternalInput").ap()
          if plan.any_dma else None)
    wt = nc.dram_tensor("wt", [F, F], f16, kind="ExternalInput").ap()
    bvec = nc.dram_tensor("bvec", [F, 1], f32, kind="ExternalInput").ap()
    wreg = nc.dram_tensor("wreg", [F, 1], f16, kind="ExternalInput").ap()
    breg = nc.dram_tensor("breg", [1, 1], f32, kind="ExternalInput").ap()
    out = nc.dram_tensor("out", [1, NPAD], f32, kind="ExternalOutput").ap()

    with tile.TileContext(nc) as tc:
        with (
            tc.tile_pool(name="const", bufs=1) as cpool,
            tc.tile_pool(name="stream", bufs=5) as spool,
            tc.tile_pool(name="ohp", bufs=6) as ogpool,
            tc.tile_pool(name="ps", bufs=6, space="PSUM") as pspool,
            tc.tile_pool(name="ph2", bufs=1, space="PSUM") as ph2pool,
            tc.tile_pool(name="po", bufs=1, space="PSUM") as popool,
            tc.tile_pool(name="hrelu", bufs=2) as hpool,
        ):
            wt_sb = cpool.tile([F, F], f16)
            b_sb = cpool.tile([F, 1], f32)
            wreg_sb = cpool.tile([F, 1], f16)
            breg_sb = cpool.tile([1, 1], f32)
            accT = cpool.tile([128, NPAD], f16)
            out_sb = cpool.tile([1, NPAD], f32)

            for sb, dr in ((wt_sb, wt), (b_sb, bvec), (wreg_sb, wreg),
                           (breg_sb, breg)):
                nc.scalar.dma_start(out=sb[:], in_=dr[:])

            if plan.any_gen:
                drl_sb = cpool.tile([128, Btot], mybir.dt.int16)
                nc.sync.dma_start(out=drl_sb[:], in_=drl[:])
                ones_sb = cpool.tile([128, 16], f16)
                nc.vector.memset(ones_sb[:], 1.0)

            # phase-2 two-stage pipeline, decoupled from the scatter stream:
            # stage A (W^T @ accT chunk) fires one bank after the chunk's
            # CAST; stage B (relu result -> cox row) one bank after stage A —
            # so each PE-queue entry's dependency is already satisfied when
            # it reaches the head (the in-order PE queue never stalls on
            # DVE/ACT work).
            pend_a = []  # chunks awaiting stage A
            pend_b = []  # (c0, c1, hr) awaiting stage B

            def phase2_a(c0, c1):
                cw = c1 - c0
                ph = ph2pool.tile([128, CH], f32)
                hr = hpool.tile([128, CH], f16)
                nc.tensor.matmul(ph[:, :cw], lhsT=wt_sb[:],
                                 rhs=accT[:, c0:c1], start=True, stop=True)
                nc.scalar.activation(hr[:, :cw], ph[:, :cw],
                                     mybir.ActivationFunctionType.Relu,
                                     bias=b_sb[:, :1])
                return hr

            out_done = [0]

            def phase2_b(c0, c1, hr):
                cw = c1 - c0
                po = popool.tile([1, CH], f32)
                nc.tensor.matmul(po[:, :cw], lhsT=wreg_sb[:], rhs=hr[:, :cw],
                                 start=True, stop=True)
                nc.scalar.activation(out_sb[:, c0:c1], po[:, :cw],
                                     mybir.ActivationFunctionType.Identity,
                                     bias=breg_sb[:, :1])
                # stream the finished half of the output row early so the
                # final out-DMA is small
                if out_done[0] == 0 and c1 >= NPAD // 2:
                    nc.scalar.dma_start(out=out[:, :c1], in_=out_sb[:, :c1])
                    out_done[0] = c1

            def pump_phase2(final=False):
                while pend_b and (final or len(pend_b) > 1):
                    phase2_b(*pend_b.pop(0))
                    if final:  # drain: issue remaining stage As first so
                        break  # they pipeline ahead of the B chain
                while pend_a and (final or len(pend_a) > 1):
                    c0, c1 = pend_a.pop(0)
                    pend_b.append((c0, c1, phase2_a(c0, c1)))
                while pend_b and final:
                    phase2_b(*pend_b.pop(0))

            done_cols = 0
            ps = None
            for gi, (k0, glen, b0, cnb) in enumerate(plan.GROUPS):
                gen = plan.gen_flags[gi]
                st = spool.tile([128, CNB_CAP * F], f16, tag="st")
                q = nc.sync
                q.dma_start(
                    out=st[:, :cnb * F].rearrange("p (c f) -> p c f", f=F),
                    in_=xg[128 * b0:128 * (b0 + cnb), :].rearrange(
                        "(p c) f -> p c f", p=128),
                )
                if gen:
                    ot = ogpool.tile([128, CNB_CAP * WN], f16, tag="ot")
                    j0 = 0
                    while j0 < cnb:
                        sub = min(16, cnb - j0)
                        nc.gpsimd.local_scatter(
                            out_ap=ot[:, j0 * WN:(j0 + sub) * WN],
                            data_ap=ones_sb[:, :sub],
                            idxs_ap=drl_sb[:, b0 + j0:b0 + j0 + sub],
                            channels=128,
                            num_elems=sub * WN,
                            num_idxs=sub,
                        )
                        j0 += sub
                else:
                    ot = ogpool.tile([128, CNB_CAP * WN], oh8, tag="ot")
                    q2 = nc.sync
                    q2.dma_start(out=ot[:, :cnb * WN],
                                 in_=oh[:, b0 * WN:(b0 + cnb) * WN])

                for i in range(glen):
                    k = k0 + i
                    nb = int(NBK[k])
                    loff = int(BB[k]) - b0
                    kb = k % BPB        # window within psum bank
                    if kb == 0:
                        ps = pspool.tile([128, CH], f32)
                    for cc in range(nb):
                        j = loff + cc
                        nc.tensor.matmul(ps[:, kb * WN:(kb + 1) * WN],
                                         lhsT=st[:, j * F:(j + 1) * F],
                                         rhs=ot[:, j * WN:(j + 1) * WN],
                                         start=(cc == 0), stop=(cc == nb - 1))
                    if kb == BPB - 1 or k == NBLK - 1:
                        cb0 = (k - kb) * WN
                        cw = (kb + 1) * WN
                        nc.vector.tensor_copy(accT[:, cb0:cb0 + cw],
                                              ps[:, :cw])
                        avail = (k + 1) * WN
                        while done_cols + CH <= avail or (k == NBLK - 1
                                                         and done_cols < NPAD):
                            c1 = min(done_cols + CH, NPAD)
                            if c1 == NPAD and c1 - done_cols > CH // 2:
                                # split the last chunk: halves the length of
                                # the final serial phase-2 dependency chain
                                mid = done_cols + (c1 - done_cols) // 2
                                pend_a.append((done_cols, mid))
                                pend_a.append((mid, c1))
                            else:
                                pend_a.append((done_cols, c1))
                            done_cols = c1
                        pump_phase2(final=(k == NBLK - 1))

            nc.sync.dma_start(out=out[:, out_done[0]:],
                              in_=out_sb[:, out_done[0]:])

    nc.compile()
    return nc


# ---------------------------------------------------------------------------
_CACHE = {}


def _ensure_ntff_hook():
    try:
        from antenv.axon_hooks import get_axon_ntff_profile_hook  # noqa: F401
        return
    except ImportError:
        pass
    import sys
    import types
    import antenv
    mod = types.ModuleType("antenv.axon_hooks")
    mod._hook = None
    mod.set_axon_ntff_profile_hook = lambda h: setattr(mod, "_hook", h)
    mod.get_axon_ntff_profile_hook = lambda: mod._hook
    sys.modules["antenv.axon_hooks"] = mod
    antenv.axon_hooks = mod
    try:
        from trn_agent_boot.trn_boot import _ntff_profile_via_ctypes
        mod._hook = _ntff_profile_via_ctypes("/opt/axon/libaxon_pjrt.so")
    except Exception:
        pass


def _run(plan, nc, trace=False):
    import concourse.bass_utils as bu
    if trace:
        _ensure_ntff_hook()
        bu.upload_artifacts = lambda tmpdir: tmpdir  # no egress here
    core_ids = list(range(len(plan.in_maps)))
    res = bu.run_bass_kernel_spmd(nc, plan.in_maps, core_ids, trace=trace)
    return res


def kernel(x, edge_index, W, b, w_reg, b_reg):
    trace = bool(os.environ.get("GCN_TRACE"))

    plan = make_plan(x, edge_index, W, b, w_reg, b_reg)
    key = (plan.B, plan.W, CNB_CAP, tuple(plan.NBK.tolist()), plan.mix,
           os.environ.get("GCN_DVE"), os.environ.get("GCN_POOL"))
    if key not in _CACHE:
        _CACHE[key] = build_nc(plan)
    nc = _CACHE[key]

    res = None
    for attempt in range(3):
        try:
            res = _run(plan, nc, trace=trace)
            break
        except Exception:
            # transient device errors (e.g. NRT exec-unit resets) recover on
            # a fresh attempt; re-raise only if persistent
            if attempt == 2:
                raise
            time.sleep(5.0)
    kernel.last_exec_ns = res.exec_time_ns
    kernel.last_profile = res.profile_json

    N = np.asarray(x).shape[0]
    ns = N // len(plan.in_maps)
    shards = [res.results[c]["out"][0, :ns] for c in range(len(plan.in_maps))]
    return np.concatenate(shards).reshape(N, 1).astype(np.float32)


kernel.last_exec_ns = None
kernel.last_profile = None
